# revision 1
# baseline (speedup 1.0000x reference)
"""CharLSTMEmbedding Trainium2 kernel.

Strategy (data-parallel over the flattened B*T=4096 word axis, 8 cores):
  - Words are globally sorted by char length (desc) and dealt round-robin to
    cores, so every core sees the same length profile (+-1 word per step).
  - At char step t only the first N_t columns (words with len > t) are
    computed; shorter words' h stays frozen in SBUF automatically.
    N_t = ceil(count(len > t) / 8) is a compile-time schedule derived from
    the actual input lengths.
  - Embedding lookup is folded into the input matmul: G = emb @ W_ih.T + bias
    (host precompute, [256 vocab, 2048]); a host-built one-hot of the char ids
    (pure index preprocessing, DMA'd per step) selects rows of G via the PE,
    adding the bias exactly once.
  - Gates are computed in [4H partition, words] orientation so h never needs
    a transpose: gates = G^T @ onehot + Whh^T @ h.
  - Matmul inputs in bf16 (fp32 PSUM accumulate); all state/activations fp32.
  - Cross-step software pipelining: each step's words are split at B = N//2;
    the bf16 h feedback lives in two ping-pong tile pairs keyed to the NEXT
    step's halves, so half-a's h-chain (ACT sigmoid/tanh + DVE) completes
    during half-b's matmuls and the next step's W-matmuls start stall-free.
  - A +-1 word ragged boundary per step is fixed with a tiny masked blend on
    the last few columns (per-core mask is input data, program stays SPMD).

kernel(**inputs) takes the full unsharded inputs and returns [32,128,512] f32.
"""

import numpy as np
import ml_dtypes

B, T, L = 32, 128, 16
VOCAB, E, H = 256, 256, 512
NCORES = 8
BT = B * T
WPC = BT // NCORES  # 512 words per core
WCAP = 16           # max blend-window width supported by the program

LAST_RESULTS = None  # test harness can read exec_time_ns from here


def _build_program(steps, blend_w, tot_ids, mask_tot):
    """steps: list of (t, N, ids_off); blend_w: dict t -> (W, mask_off)."""
    import concourse.bass as bass
    import concourse.tile as tile
    from concourse import bacc, mybir
    from contextlib import ExitStack

    f32 = mybir.dt.float32
    bf16 = mybir.dt.bfloat16
    AF = mybir.ActivationFunctionType
    ALU = mybir.AluOpType

    nc = bacc.Bacc("TRN2", target_bir_lowering=False, debug=False)

    g_d = nc.dram_tensor("g", [128, 2 * 2048], bf16, kind="ExternalInput")
    whh_d = nc.dram_tensor("whh", [128, 4 * 2048], bf16, kind="ExternalInput")
    oh_d = nc.dram_tensor("oh", [128, 2 * tot_ids], bf16, kind="ExternalInput")
    if mask_tot > 0:
        mask_d = nc.dram_tensor("mask", [128, mask_tot], f32, kind="ExternalInput")
    hout_d = nc.dram_tensor("h_out", [128, 2048], f32, kind="ExternalOutput")

    with tile.TileContext(nc) as tc, ExitStack() as ctx:
        # persistent tensors (one bufs=1 pool, distinct tags -> distinct slots)
        cpool = ctx.enter_context(tc.tile_pool(name="const", bufs=1))
        g_sb = cpool.tile([128, 2, 2048], bf16, name="g_sb", tag="g_sb")
        whh_sb = cpool.tile([128, 4, 2048], bf16, name="whh_sb", tag="whh_sb")
        if mask_tot > 0:
            mask_sb = cpool.tile([128, mask_tot], f32, name="mask_sb", tag="mask_sb")
        h_sb = cpool.tile([128, 4, 512], f32, name="h_sb", tag="h_sb")
        hbfA = [
            cpool.tile([128, 4, 256], bf16, name=f"hbfA{j}", tag=f"hbfA{j}")
            for j in range(2)
        ]
        hbfB = [
            cpool.tile([128, 4, 256], bf16, name=f"hbfB{j}", tag=f"hbfB{j}")
            for j in range(2)
        ]
        c_sb = cpool.tile([128, 4, 512], f32, name="c_sb", tag="c_sb")

        nc.sync.dma_start(g_sb[:, :, :], g_d.rearrange("p (v m) -> p v m", v=2))
        nc.vector.memset(h_sb[:, :, :], 0.0)
        warm = cpool.tile([128, 8], f32, name="warm", tag="warm")
        nc.vector.memset(warm[:, :], 0.0)
        nc.scalar.activation(warm[:, :], warm[:, :], AF.Sigmoid)

        # rotating pools
        gate_pool = ctx.enter_context(tc.tile_pool(name="gps", bufs=2, space="PSUM"))
        oh_pool = ctx.enter_context(tc.tile_pool(name="oh", bufs=3))
        act_pool = ctx.enter_context(tc.tile_pool(name="acts", bufs=1))
        tmp_pool = ctx.enter_context(tc.tile_pool(name="tmps", bufs=1))
        bl_pool = ctx.enter_context(tc.tile_pool(name="blend", bufs=2))

        n_steps = len(steps)
        emitted_hi_dma = [False]
        for si, (t, N, off) in enumerate(steps):
            first = si == 0
            last = si == n_steps - 1
            split = N > 220
            Bs = N // 2 if split else N          # this step's half boundary
            rA, rB = hbfA[si % 2], hbfB[si % 2]          # read set
            wA, wB = hbfA[(si + 1) % 2], hbfB[(si + 1) % 2]  # write set
            if not last:
                Nn = steps[si + 1][1]            # next step's width/boundary
                Bn = Nn // 2 if Nn > 220 else Nn
            halves = [(0, Bs)] + ([(Bs, N)] if split else [])

            oh = oh_pool.tile([128, 2, 512], bf16, name=f"oh{t}", tag="oh")
            nc.sync.dma_start(
                oh[:, :, :N],
                oh_d[:, 2 * off : 2 * (off + N)].rearrange("p (v n) -> p v n", v=2),
            )
            if first:
                # deferred so step 0's inputs win the HBM bandwidth race
                nc.sync.dma_start(
                    whh_sb[:, :, :], whh_d.rearrange("p (k m) -> p k m", k=4)
                )
                if mask_tot > 0:
                    nc.sync.dma_start(mask_sb[:, :], mask_d[:, :])

            if si > 0 and steps[si - 1][1] > 256 and N <= 256:
                emitted_hi_dma[0] = True
                # columns [256:512) are final now; stream them out early
                nc.sync.dma_start(
                    hout_d.rearrange("p (j n) -> p j n", j=4)[:, :, 256:],
                    h_sb[:, :, 256:],
                )
            W, moff = blend_w.get(t, (0, 0))
            for hi, (s, e) in enumerate(halves):
                n = e - s

                def emit_group(grp):
                    # G-phase (independent of h) for all m-tiles, then W-phase
                    ps = gate_pool.tile(
                        [128, 4, 512], f32, name=f"ps{grp}_{t}_{hi}", tag="ps"
                    )
                    for m4 in range(4):
                        m = grp * 4 + m4
                        nc.tensor.matmul(
                            ps[:, m4, :n], g_sb[:, 0, m * 128 : (m + 1) * 128],
                            oh[:, 0, s:e], start=True, stop=False,
                        )
                        nc.tensor.matmul(
                            ps[:, m4, :n], g_sb[:, 1, m * 128 : (m + 1) * 128],
                            oh[:, 1, s:e], start=False, stop=first,
                        )
                    if not first:
                        for m4 in range(4):
                            m = grp * 4 + m4
                            for kk in range(4):
                                if e <= Bs:
                                    rhs = rA[:, kk, s:e]
                                else:
                                    rhs = rB[:, kk, s - Bs : e - Bs]
                                nc.tensor.matmul(
                                    ps[:, m4, :n],
                                    whh_sb[:, kk, m * 128 : (m + 1) * 128],
                                    rhs, start=False, stop=(kk == 3),
                                )
                    at = act_pool.tile(
                        [128, 4, 256], f32,
                        name=f"a{grp}_{t}_{hi}", tag=f"a{grp}{hi}",
                    )
                    func = AF.Tanh if grp == 2 else AF.Sigmoid
                    nc.scalar.activation(at[:, :, :n], ps[:, :, :n], func)
                    return at

                # i, f, g first; c and tanh(c) run while o's matmuls execute,
                # keeping tanh(c) ahead of o's activation in the ACT FIFO.
                it = emit_group(0)
                ft = emit_group(1)
                gt = emit_group(2)
                if first:
                    nc.vector.tensor_mul(
                        c_sb[:, :, s:e], it[:, :, :n], gt[:, :, :n]
                    )
                else:
                    ig = tmp_pool.tile(
                        [128, 4, 256], f32, name=f"ig{t}_{hi}", tag=f"ig{hi}"
                    )
                    nc.vector.tensor_mul(ig[:, :, :n], it[:, :, :n], gt[:, :, :n])
                    nc.vector.tensor_mul(
                        c_sb[:, :, s:e], ft[:, :, :n], c_sb[:, :, s:e]
                    )
                    nc.vector.tensor_add(
                        c_sb[:, :, s:e], c_sb[:, :, s:e], ig[:, :, :n]
                    )
                th = tmp_pool.tile(
                    [128, 4, 256], f32, name=f"th{t}_{hi}", tag=f"th{hi}"
                )
                nc.scalar.activation(th[:, :, :n], c_sb[:, :, s:e], AF.Tanh)

                ot = emit_group(3)

                # critical path: bf16 h tiles keyed to the NEXT step's halves
                if not last:
                    lo, hi_ = s, min(e, Bn)
                    if lo < hi_:
                        nc.vector.tensor_mul(
                            wA[:, :, lo:hi_],
                            ot[:, :, lo - s : hi_ - s], th[:, :, lo - s : hi_ - s],
                        )
                    lo, hi_ = max(s, Bn), min(e, Nn)
                    if lo < hi_:
                        nc.vector.tensor_mul(
                            wB[:, :, lo - Bn : hi_ - Bn],
                            ot[:, :, lo - s : hi_ - s], th[:, :, lo - s : hi_ - s],
                        )

                # off critical path: fp32 h (output state) + boundary blend
                wlo = min(e, max(s, N - W)) if W > 0 else e
                if wlo > s:
                    nc.vector.tensor_mul(
                        h_sb[:, :, s:wlo], ot[:, :, : wlo - s], th[:, :, : wlo - s]
                    )
                if wlo < e:
                    bw = e - wlo
                    mlo = wlo - (N - W)
                    hw = bl_pool.tile(
                        [128, 4, WCAP], f32, name=f"hw{t}_{hi}", tag="hw"
                    )
                    nc.vector.tensor_mul(
                        hw[:, :, :bw], ot[:, :, wlo - s : e - s],
                        th[:, :, wlo - s : e - s],
                    )
                    mview = mask_sb[:, moff : moff + 4 * W].rearrange(
                        "p (j w) -> p j w", j=4
                    )
                    # h_win = h_new + minv*(h_old - h_new), minv=1 frozen
                    dd = bl_pool.tile(
                        [128, 4, WCAP], f32, name=f"dd{t}_{hi}", tag="dd"
                    )
                    nc.vector.tensor_sub(
                        dd[:, :, :bw], h_sb[:, :, wlo:e], hw[:, :, :bw]
                    )
                    nc.vector.tensor_mul(
                        dd[:, :, :bw], dd[:, :, :bw], mview[:, :, mlo : mlo + bw]
                    )
                    nc.vector.tensor_add(
                        h_sb[:, :, wlo:e], hw[:, :, :bw], dd[:, :, :bw]
                    )

        if not emitted_hi_dma[0]:
            nc.sync.dma_start(
                hout_d.rearrange("p (j n) -> p j n", j=4)[:, :, 256:],
                h_sb[:, :, 256:],
            )
        nc.sync.dma_start(
            hout_d.rearrange("p (j n) -> p j n", j=4)[:, :, :256], h_sb[:, :, :256]
        )

    nc.compile()
    return nc


def kernel(char_seq_padded, char_lengths, emb, W_ih, W_hh, b_ih, b_hh):
    global LAST_RESULTS
    from concourse.bass_utils import run_bass_kernel_spmd

    char_seq_padded = np.asarray(char_seq_padded)
    in_dtype = char_seq_padded.dtype
    ids_all = char_seq_padded.reshape(BT, L)
    lens = np.asarray(char_lengths).reshape(BT).astype(np.int64)
    emb = np.asarray(emb, dtype=np.float32)
    W_ih = np.asarray(W_ih, dtype=np.float32)
    W_hh = np.asarray(W_hh, dtype=np.float32)
    bias = np.asarray(b_ih, dtype=np.float32) + np.asarray(b_hh, dtype=np.float32)

    # ---- host precompute ----
    G = (emb @ W_ih.T + bias).astype(np.float32)  # [VOCAB, 4H]
    WhhT = np.ascontiguousarray(W_hh.T)           # [H, 4H]
    g_dev = np.ascontiguousarray(
        G.reshape(2, 128, 4 * H).transpose(1, 0, 2).reshape(128, 2 * 4 * H)
    ).astype(ml_dtypes.bfloat16)
    whh_dev = np.ascontiguousarray(
        WhhT.reshape(4, 128, 4 * H).transpose(1, 0, 2).reshape(128, 4 * 4 * H)
    ).astype(ml_dtypes.bfloat16)
    # ---- ragged schedule ----
    order = np.argsort(-lens, kind="stable")
    perms = [order[k::NCORES] for k in range(NCORES)]      # each [WPC], len-desc
    cnts = np.stack(
        [(lens[p][:, None] > np.arange(L)[None, :]).sum(0) for p in perms]
    )  # [NCORES, L]
    C = (lens[:, None] > np.arange(L)[None, :]).sum(0)     # [L] global counts

    steps = []      # (t, N, ids_off)
    blend_w = {}    # t -> (W, mask_off)
    off = 0
    moff = 0
    ids_core = [[] for _ in range(NCORES)]
    mask_core = [[] for _ in range(NCORES)]
    for t in range(L):
        if C[t] == 0:
            continue
        N = int(-(-C[t] // NCORES))  # ceil
        steps.append((t, N, off))
        off += N
        vocab_col = np.arange(VOCAB, dtype=np.int32)[:, None]
        for k in range(NCORES):
            ids_t = ids_all[perms[k][:N], t]  # [N]
            one_hot = (ids_t[None, :] == vocab_col)  # [VOCAB, N]
            # device layout [128 partitions, (v, n)]: partition p, tile v -> vocab v*128+p
            oh_dev = one_hot.reshape(2, 128, N).transpose(1, 0, 2).reshape(128, 2 * N)
            ids_core[k].append(oh_dev.astype(ml_dtypes.bfloat16))
        W = int(N - cnts[:, t].min())
        if W > 0:
            assert W <= WCAP
            blend_w[t] = (W, moff)
            moff += 4 * W
            for k in range(NCORES):
                # inverted: 1.0 = frozen word (keep old h), 0.0 = active
                m = (np.arange(N - W, N) >= cnts[k, t]).astype(np.float32)
                mask_core[k].append(np.tile(m, 4))
    tot_ids = off
    mask_tot = moff

    nc = _build_program(steps, blend_w, tot_ids, mask_tot)

    in_maps = []
    for k in range(NCORES):
        m = {
            "g": g_dev,
            "whh": whh_dev,
            "oh": np.ascontiguousarray(np.concatenate(ids_core[k], axis=1)),
        }
        if mask_tot > 0:
            mrow = np.concatenate(mask_core[k])[None, :]  # [1, mask_tot]
            m["mask"] = np.ascontiguousarray(np.repeat(mrow, 128, axis=0))
        in_maps.append(m)

    res = run_bass_kernel_spmd(nc, in_maps, list(range(NCORES)))
    LAST_RESULTS = res

    out = np.empty((BT, H), dtype=np.float32)
    for k in range(NCORES):
        hk = res.results[k]["h_out"]  # [128, 2048]
        out[perms[k]] = hk.reshape(128, 4, 512).transpose(2, 1, 0).reshape(WPC, H)
    return out.reshape(B, T, H)



# revision 2
# speedup vs baseline: 1.2561x; 1.2561x over previous
"""CharLSTMEmbedding Trainium2 kernel (v2: fp8 DoubleRow hidden matmul).

Strategy (data-parallel over the flattened B*T=4096 word axis, 8 cores):
  - Words are globally sorted by char length (desc) and dealt round-robin to
    cores, so every core sees the same length profile (+-1 word per step).
  - At char step t only the first N_t columns (words with len > t) are
    computed; shorter words' h stays frozen in SBUF automatically.
    N_t = ceil(count(len > t) / 8) is a compile-time schedule derived from
    the actual input lengths.
  - Embedding lookup is folded into the input matmul: G = (emb @ W_ih.T +
    bias) * S (host precompute, bf16, [256 vocab, 2048]); a host-built
    one-hot of the char ids selects rows of G via the PE in 2 bf16 matmuls.
  - Hidden matmul runs in fp8e4 with perf_mode=DoubleRow (2 instructions of
    K=256 each instead of 4 bf16 K=128): Whh is host-quantized to
    e4m3(Whh * S) with S = 120/absmax(Whh); the h feedback tiles are e4m3
    (values in [-1,1], written directly by the DVE). Both matmul groups
    accumulate into the same PSUM at scale S; the gate activations descale
    with the ACT input-scale (sigmoid/tanh(psum/S)) for free.
  - Gate activations + tanh(c) output bf16 and the whole c chain (ig, f*c,
    +) runs on 2-byte operands, enabling the DVE 2x/4x fast modes; c state
    is bf16. The fp32 h output write is narrowed per step to the
    "delta window" [min_k cnts_k(t+1), N_t) - only words that finish at
    step t need their final h persisted (plus the masked boundary blend).
  - Cross-step software pipelining: each step's words are split at B = N//2;
    the e4m3 h feedback lives in two ping-pong tile pairs keyed to the NEXT
    step's halves, so half-a's h-chain (ACT sigmoid/tanh + DVE) completes
    during half-b's matmuls and the next step's W-matmuls start stall-free.
  - A +-1 word ragged boundary per step is fixed with a tiny masked blend on
    the last few columns (per-core mask is input data, program stays SPMD).

kernel(**inputs) takes the full unsharded inputs and returns [32,128,512] f32.
"""

import numpy as np
import ml_dtypes

B, T, L = 32, 128, 16
VOCAB, E, H = 256, 256, 512
NCORES = 8
BT = B * T
WPC = BT // NCORES  # 512 words per core
WCAP = 16           # max blend-window width supported by the program

LAST_RESULTS = None  # test harness can read exec_time_ns from here


def _build_program(steps, blend_w, tot_ids, mask_tot, inv_s):
    """steps: list of (t, N, ids_off, Nlo); blend_w: dict t -> (W, mask_off).
    inv_s: 1/S descale applied via the gate activations' input scale."""
    import concourse.bass as bass
    import concourse.tile as tile
    from concourse import bacc, mybir
    from contextlib import ExitStack

    f32 = mybir.dt.float32
    bf16 = mybir.dt.bfloat16
    fp8 = mybir.dt.float8e4
    AF = mybir.ActivationFunctionType
    ALU = mybir.AluOpType
    DR = mybir.MatmulPerfMode.DoubleRow

    nc = bacc.Bacc("TRN2", target_bir_lowering=False, debug=False)

    g_d = nc.dram_tensor("g", [128, 2 * 2048], bf16, kind="ExternalInput")
    whh_d = nc.dram_tensor("whh", [128, 4 * 2048], fp8, kind="ExternalInput")
    oh_d = nc.dram_tensor("oh", [128, 2 * tot_ids], bf16, kind="ExternalInput")
    if mask_tot > 0:
        mask_d = nc.dram_tensor("mask", [128, mask_tot], f32, kind="ExternalInput")
    hout_d = nc.dram_tensor("h_out", [128, 2048], f32, kind="ExternalOutput")

    with tile.TileContext(nc) as tc, ExitStack() as ctx:
        # persistent tensors (one bufs=1 pool, distinct tags -> distinct slots)
        cpool = ctx.enter_context(tc.tile_pool(name="const", bufs=1))
        g_sb = cpool.tile([128, 2, 2048], bf16, name="g_sb", tag="g_sb")
        whh_sb = cpool.tile([128, 4, 2048], fp8, name="whh_sb", tag="whh_sb")
        if mask_tot > 0:
            mask_sb = cpool.tile([128, mask_tot], f32, name="mask_sb", tag="mask_sb")
        h_sb = cpool.tile([128, 4, 512], f32, name="h_sb", tag="h_sb")
        hbfA = [
            cpool.tile([128, 4, 256], fp8, name=f"hbfA{j}", tag=f"hbfA{j}")
            for j in range(2)
        ]
        hbfB = [
            cpool.tile([128, 4, 256], fp8, name=f"hbfB{j}", tag=f"hbfB{j}")
            for j in range(2)
        ]
        c_sb = cpool.tile([128, 4, 512], bf16, name="c_sb", tag="c_sb")

        nc.sync.dma_start(g_sb[:, :, :], g_d.rearrange("p (v m) -> p v m", v=2))
        nc.vector.memset(h_sb[:, :, :], 0.0)
        warm = cpool.tile([128, 8], f32, name="warm", tag="warm")
        nc.vector.memset(warm[:, :], 0.0)
        nc.scalar.activation(warm[:, :], warm[:, :], AF.Sigmoid)

        # rotating pools
        gate_pool = ctx.enter_context(tc.tile_pool(name="gps", bufs=2, space="PSUM"))
        oh_pool = ctx.enter_context(tc.tile_pool(name="oh", bufs=3))
        act_pool = ctx.enter_context(tc.tile_pool(name="acts", bufs=1))
        tmp_pool = ctx.enter_context(tc.tile_pool(name="tmps", bufs=1))
        bl_pool = ctx.enter_context(tc.tile_pool(name="blend", bufs=2))

        n_steps = len(steps)
        emitted_hi_dma = [False]
        for si, (t, N, off, Nlo) in enumerate(steps):
            first = si == 0
            last = si == n_steps - 1
            split = N > 220
            Bs = N // 2 if split else N          # this step's half boundary
            rA, rB = hbfA[si % 2], hbfB[si % 2]          # read set
            wA, wB = hbfA[(si + 1) % 2], hbfB[(si + 1) % 2]  # write set
            if not last:
                Nn = steps[si + 1][1]            # next step's width/boundary
                Bn = Nn // 2 if Nn > 220 else Nn
            halves = [(0, Bs)] + ([(Bs, N)] if split else [])

            oh = oh_pool.tile([128, 2, 512], bf16, name=f"oh{t}", tag="oh")
            nc.sync.dma_start(
                oh[:, :, :N],
                oh_d[:, 2 * off : 2 * (off + N)].rearrange("p (v n) -> p v n", v=2),
            )
            if first:
                # deferred so step 0's inputs win the HBM bandwidth race
                nc.sync.dma_start(
                    whh_sb[:, :, :], whh_d.rearrange("p (k m) -> p k m", k=4)
                )
                if mask_tot > 0:
                    nc.sync.dma_start(mask_sb[:, :], mask_d[:, :])

            if si > 0 and steps[si - 1][1] > 256 and N <= 256:
                emitted_hi_dma[0] = True
                # columns [256:512) are final now; stream them out early
                nc.sync.dma_start(
                    hout_d.rearrange("p (j n) -> p j n", j=4)[:, :, 256:],
                    h_sb[:, :, 256:],
                )
            W, moff = blend_w.get(t, (0, 0))
            for hi, (s, e) in enumerate(halves):
                n = e - s

                def emit_group(grp):
                    # G-phase (independent of h) for all m-tiles, then W-phase
                    ps = gate_pool.tile(
                        [128, 4, 512], f32, name=f"ps{grp}_{t}_{hi}", tag="ps"
                    )
                    for m4 in range(4):
                        m = grp * 4 + m4
                        nc.tensor.matmul(
                            ps[:, m4, :n], g_sb[:, 0, m * 128 : (m + 1) * 128],
                            oh[:, 0, s:e], start=True, stop=False,
                        )
                        nc.tensor.matmul(
                            ps[:, m4, :n], g_sb[:, 1, m * 128 : (m + 1) * 128],
                            oh[:, 1, s:e], start=False, stop=first,
                        )
                    if not first:
                        for m4 in range(4):
                            m = grp * 4 + m4
                            for kp in range(2):
                                if e <= Bs:
                                    rhs = rA[:, 2 * kp : 2 * kp + 2, s:e]
                                else:
                                    rhs = rB[:, 2 * kp : 2 * kp + 2, s - Bs : e - Bs]
                                nc.tensor.matmul(
                                    ps[:, m4, :n],
                                    whh_sb[:, 2 * kp : 2 * kp + 2, m * 128 : (m + 1) * 128],
                                    rhs, start=False, stop=(kp == 1),
                                    perf_mode=DR,
                                )
                    at = act_pool.tile(
                        [128, 4, 256], bf16,
                        name=f"a{grp}_{t}_{hi}", tag=f"a{grp}{hi}",
                    )
                    func = AF.Tanh if grp == 2 else AF.Sigmoid
                    nc.scalar.activation(at[:, :, :n], ps[:, :, :n], func,
                                         scale=inv_s)
                    return at

                # i, f, g first; c and tanh(c) run while o's matmuls execute,
                # keeping tanh(c) ahead of o's activation in the ACT FIFO.
                it = emit_group(0)
                ft = emit_group(1)
                gt = emit_group(2)
                if first:
                    nc.vector.tensor_mul(
                        c_sb[:, :, s:e], it[:, :, :n], gt[:, :, :n]
                    )
                else:
                    ig = tmp_pool.tile(
                        [128, 4, 256], bf16, name=f"ig{t}_{hi}", tag=f"ig{hi}"
                    )
                    nc.vector.tensor_mul(ig[:, :, :n], it[:, :, :n], gt[:, :, :n])
                    nc.vector.tensor_mul(
                        c_sb[:, :, s:e], ft[:, :, :n], c_sb[:, :, s:e]
                    )
                    nc.vector.tensor_add(
                        c_sb[:, :, s:e], c_sb[:, :, s:e], ig[:, :, :n]
                    )
                th = tmp_pool.tile(
                    [128, 4, 256], bf16, name=f"th{t}_{hi}", tag=f"th{hi}"
                )
                nc.scalar.activation(th[:, :, :n], c_sb[:, :, s:e], AF.Tanh)

                ot = emit_group(3)

                # critical path: e4m3 h tiles keyed to the NEXT step's halves
                if not last:
                    lo, hi_ = s, min(e, Bn)
                    if lo < hi_:
                        nc.vector.tensor_mul(
                            wA[:, :, lo:hi_],
                            ot[:, :, lo - s : hi_ - s], th[:, :, lo - s : hi_ - s],
                        )
                    lo, hi_ = max(s, Bn), min(e, Nn)
                    if lo < hi_:
                        nc.vector.tensor_mul(
                            wB[:, :, lo - Bn : hi_ - Bn],
                            ot[:, :, lo - s : hi_ - s], th[:, :, lo - s : hi_ - s],
                        )

                # off critical path: fp32 h (output state), only the columns
                # that can finish at this step (delta window), + boundary blend
                wlo = min(e, max(s, N - W)) if W > 0 else e
                dlo = max(s, min(Nlo, wlo))
                if wlo > dlo:
                    nc.vector.tensor_mul(
                        h_sb[:, :, dlo:wlo],
                        ot[:, :, dlo - s : wlo - s], th[:, :, dlo - s : wlo - s],
                    )
                if wlo < e:
                    bw = e - wlo
                    mlo = wlo - (N - W)
                    hw = bl_pool.tile(
                        [128, 4, WCAP], f32, name=f"hw{t}_{hi}", tag="hw"
                    )
                    nc.vector.tensor_mul(
                        hw[:, :, :bw], ot[:, :, wlo - s : e - s],
                        th[:, :, wlo - s : e - s],
                    )
                    mview = mask_sb[:, moff : moff + 4 * W].rearrange(
                        "p (j w) -> p j w", j=4
                    )
                    # h_win = h_new + minv*(h_old - h_new), minv=1 frozen
                    dd = bl_pool.tile(
                        [128, 4, WCAP], f32, name=f"dd{t}_{hi}", tag="dd"
                    )
                    nc.vector.tensor_sub(
                        dd[:, :, :bw], h_sb[:, :, wlo:e], hw[:, :, :bw]
                    )
                    nc.vector.tensor_mul(
                        dd[:, :, :bw], dd[:, :, :bw], mview[:, :, mlo : mlo + bw]
                    )
                    nc.vector.tensor_add(
                        h_sb[:, :, wlo:e], hw[:, :, :bw], dd[:, :, :bw]
                    )

        if not emitted_hi_dma[0]:
            nc.sync.dma_start(
                hout_d.rearrange("p (j n) -> p j n", j=4)[:, :, 256:],
                h_sb[:, :, 256:],
            )
        nc.sync.dma_start(
            hout_d.rearrange("p (j n) -> p j n", j=4)[:, :, :256], h_sb[:, :, :256]
        )

    nc.compile()
    return nc


def kernel(char_seq_padded, char_lengths, emb, W_ih, W_hh, b_ih, b_hh):
    global LAST_RESULTS
    from concourse.bass_utils import run_bass_kernel_spmd

    char_seq_padded = np.asarray(char_seq_padded)
    ids_all = char_seq_padded.reshape(BT, L)
    lens = np.asarray(char_lengths).reshape(BT).astype(np.int64)
    emb = np.asarray(emb, dtype=np.float32)
    W_ih = np.asarray(W_ih, dtype=np.float32)
    W_hh = np.asarray(W_hh, dtype=np.float32)
    bias = np.asarray(b_ih, dtype=np.float32) + np.asarray(b_hh, dtype=np.float32)

    # ---- host precompute ----
    s_w = float(120.0 / np.abs(W_hh).max())     # fp8 scale; PSUM carries S
    G = ((emb @ W_ih.T + bias) * s_w).astype(np.float32)  # [VOCAB, 4H] * S
    WhhT = np.ascontiguousarray(W_hh.T * s_w)   # [H, 4H] * S
    g_dev = np.ascontiguousarray(
        G.reshape(2, 128, 4 * H).transpose(1, 0, 2).reshape(128, 2 * 4 * H)
    ).astype(ml_dtypes.bfloat16)
    whh_dev = np.ascontiguousarray(
        WhhT.reshape(4, 128, 4 * H).transpose(1, 0, 2).reshape(128, 4 * 4 * H)
    ).astype(ml_dtypes.float8_e4m3)
    # ---- ragged schedule ----
    order = np.argsort(-lens, kind="stable")
    perms = [order[k::NCORES] for k in range(NCORES)]      # each [WPC], len-desc
    cnts = np.stack(
        [(lens[p][:, None] > np.arange(L)[None, :]).sum(0) for p in perms]
    )  # [NCORES, L]
    C = (lens[:, None] > np.arange(L)[None, :]).sum(0)     # [L] global counts

    steps = []      # (t, N, ids_off, Nlo)
    blend_w = {}    # t -> (W, mask_off)
    off = 0
    moff = 0
    ids_core = [[] for _ in range(NCORES)]
    mask_core = [[] for _ in range(NCORES)]
    tlist = [t for t in range(L) if C[t] > 0]
    for ti, t in enumerate(tlist):
        N = int(-(-C[t] // NCORES))  # ceil
        # delta window: only columns that can end at step t need the fp32
        # h write; col j ends at t iff j >= cnts_k(t+1) on its core
        if ti == len(tlist) - 1:
            Nlo = 0
        else:
            Nlo = int(cnts[:, tlist[ti + 1]].min())
        steps.append((t, N, off, Nlo))
        off += N
        vocab_col = np.arange(VOCAB, dtype=np.int32)[:, None]
        for k in range(NCORES):
            ids_t = ids_all[perms[k][:N], t]  # [N]
            one_hot = (ids_t[None, :] == vocab_col)  # [VOCAB, N]
            # device layout [128 partitions, (v, n)]: partition p, tile v -> vocab v*128+p
            oh_dev = one_hot.reshape(2, 128, N).transpose(1, 0, 2).reshape(128, 2 * N)
            ids_core[k].append(oh_dev.astype(ml_dtypes.bfloat16))
        W = int(N - cnts[:, t].min())
        if W > 0:
            assert W <= WCAP
            blend_w[t] = (W, moff)
            moff += 4 * W
            for k in range(NCORES):
                # inverted: 1.0 = frozen word (keep old h), 0.0 = active
                m = (np.arange(N - W, N) >= cnts[k, t]).astype(np.float32)
                mask_core[k].append(np.tile(m, 4))
    tot_ids = off
    mask_tot = moff

    nc = _build_program(steps, blend_w, tot_ids, mask_tot, 1.0 / s_w)

    in_maps = []
    for k in range(NCORES):
        m = {
            "g": g_dev,
            "whh": whh_dev,
            "oh": np.ascontiguousarray(np.concatenate(ids_core[k], axis=1)),
        }
        if mask_tot > 0:
            mrow = np.concatenate(mask_core[k])[None, :]  # [1, mask_tot]
            m["mask"] = np.ascontiguousarray(np.repeat(mrow, 128, axis=0))
        in_maps.append(m)

    res = run_bass_kernel_spmd(nc, in_maps, list(range(NCORES)))
    LAST_RESULTS = res

    out = np.empty((BT, H), dtype=np.float32)
    for k in range(NCORES):
        hk = res.results[k]["h_out"]  # [128, 2048]
        out[perms[k]] = hk.reshape(128, 4, 512).transpose(2, 1, 0).reshape(WPC, H)
    return out.reshape(B, T, H)
